# revision 4
# baseline (speedup 1.0000x reference)
"""GRPO fused-linear loss kernel for 8 Trainium2 NeuronCores.

Strategy (vocab-tensor-parallel):
  - The only heavy math is two fused-linear-logsumexp passes:
    logits = x @ W^T over [4096 tokens, 1024 hidden] x [32000 vocab, 1024].
    Per-token we need only logsumexp(logits) and the logit at the selected
    token id.  exp(x - x) == 1.0 exactly in IEEE fp so the PPO ratio terms
    collapse: per_token_loss = -advantage, clip_ratio = 0.
  - Vocab is padded to 32768 and split 4096 rows per core.  Each core
    computes partial sum(exp(logit)) per token over its vocab shard
    (no max-subtraction needed: logits are ~N(0, 0.013), exp is safe).
  - Selected-token logits are computed separately as row-dot-products
    x[t] . W[id_t] with host-gathered weight rows, token-sharded 512/core.
  - Host combines: logsumexp across shards (minus the zero-pad columns'
    exp(0) contribution), percentile threshold, masked k3 KL, final scalars.

Device layout per core (all weights/activations pre-transposed on host so
H sits on SBUF partitions; no on-device transposes):
  xT  [1024, 4096]  x^T (replicated)         wT  [1024, 4096] W^T vocab shard
  rxT [1024, 4096]  ref x^T (replicated)     rwT [1024, 4096] ref W^T shard
  xs  [512, 1024]   x rows, token shard      ws  [512, 1024]  W[id] rows
  rxs [512, 1024]                            rws [512, 1024]
Outputs:
  se  [2, 128, 32]  partial sumexp, token t = t_tile*128 + p
  sel [2, 128, 4]   selected logit for local token lt = st*128 + p

Matmul dtype: float32r (FP22 multiplies, full PE rate at N=512) by default,
or bfloat16 (host-converted inputs).
"""

import numpy as np

import concourse.bass as bass  # noqa: F401  (bass types used indirectly)
import concourse.mybir as mybir
import concourse.tile as tile
from concourse import bacc
from concourse.bass_utils import run_bass_kernel_spmd

B, T, H, V = 8, 512, 1024, 32000
TOK = B * T              # 4096 tokens
NCORE = 8
VP = 4096                # padded vocab rows per core (8*4096 = 32768)
TSH = TOK // NCORE       # 512 tokens per core for the selected-logit dots
NV = 512                 # vocab chunk per psum tile
HC = H // 128            # 8 hidden chunks
VHALF = VP // 2          # vocab half resident in SBUF at a time

BETA = 0.04
EPS_LOW = 0.2
EPS_HIGH = 0.2
KL_PERCENTILE = 0.2

MM_DTYPE = "f32r"        # "f32r" | "bf16"

_nc_cache = {}


def build_nc(mm_dtype=MM_DTYPE):
    if mm_dtype in _nc_cache:
        return _nc_cache[mm_dtype]
    dt = mybir.dt
    f32 = dt.float32
    mmdt = dt.float32r if mm_dtype == "f32r" else dt.bfloat16

    nc = bacc.Bacc("TRN2", target_bir_lowering=False, debug=False,
                   num_devices=NCORE)

    xT = nc.dram_tensor("xT", [H, TOK], mmdt, kind="ExternalInput")
    rxT = nc.dram_tensor("rxT", [H, TOK], mmdt, kind="ExternalInput")
    wT = nc.dram_tensor("wT", [H, VP], mmdt, kind="ExternalInput")
    rwT = nc.dram_tensor("rwT", [H, VP], mmdt, kind="ExternalInput")
    xs = nc.dram_tensor("xs", [TSH, H], f32, kind="ExternalInput")
    rxs = nc.dram_tensor("rxs", [TSH, H], f32, kind="ExternalInput")
    ws = nc.dram_tensor("ws", [TSH, H], f32, kind="ExternalInput")
    rws = nc.dram_tensor("rws", [TSH, H], f32, kind="ExternalInput")
    se = nc.dram_tensor("se", [2, 128, TOK // 128], f32, kind="ExternalOutput")
    sel = nc.dram_tensor("sel", [2, 128, TSH // 128], f32, kind="ExternalOutput")

    ttiles = TOK // 128          # 32
    gsz = 512                    # tokens per streamed x group
    ngroups = TOK // gsz         # 8
    nvchunk = VHALF // NV        # 4 vocab chunks per half

    def mm_ap(ap):
        return ap

    with tile.TileContext(nc) as tc:
        with (
            tc.tile_pool(name="wt", bufs=2) as wt_pool,
            tc.tile_pool(name="xg", bufs=2) as xg_pool,
            tc.tile_pool(name="ps", bufs=4, space="PSUM") as ps_pool,
            tc.tile_pool(name="sc", bufs=2) as sc_pool,
            tc.tile_pool(name="acc", bufs=4) as acc_pool,
            tc.tile_pool(name="outs", bufs=2) as out_pool,
            tc.tile_pool(name="selp", bufs=1) as sel_pool,
        ):
            for m, (xT_d, wT_d, xs_d, ws_d) in enumerate(
                [(xT, wT, xs, ws), (rxT, rwT, rxs, rws)]
            ):
                se_halves = []
                for half in range(2):
                    # resident half of the vocab-shard weights [128, 8, 2048]
                    wt_t = wt_pool.tile([128, HC, VHALF], mmdt, tag="wt")
                    for hc in range(HC):
                        nc.sync.dma_start(
                            wt_t[:, hc, :],
                            wT_d.ap()[hc * 128:(hc + 1) * 128,
                                      half * VHALF:(half + 1) * VHALF],
                        )
                    se_h = out_pool.tile([128, ttiles], f32, tag="se_h")
                    for g in range(ngroups):
                        xg_t = xg_pool.tile([128, HC, gsz], mmdt, tag="xg")
                        for hc in range(HC):
                            nc.sync.dma_start(
                                xg_t[:, hc, :],
                                xT_d.ap()[hc * 128:(hc + 1) * 128,
                                          g * gsz:(g + 1) * gsz],
                            )
                        for tt in range(gsz // 128):
                            gtt = g * (gsz // 128) + tt
                            acc_t = acc_pool.tile([128, nvchunk], f32, tag="acc")
                            for v in range(nvchunk):
                                ps = ps_pool.tile([128, NV], f32, tag="ps")
                                for hc in range(HC):
                                    nc.tensor.matmul(
                                        ps[:],
                                        mm_ap(xg_t[:, hc, tt * 128:(tt + 1) * 128]),
                                        mm_ap(wt_t[:, hc, v * NV:(v + 1) * NV]),
                                        start=(hc == 0),
                                        stop=(hc == HC - 1),
                                    )
                                sc = sc_pool.tile([128, NV], f32, tag="sc")
                                nc.scalar.activation(
                                    out=sc[:],
                                    in_=ps[:],
                                    func=mybir.ActivationFunctionType.Exp,
                                    accum_out=acc_t[:, v:v + 1],
                                )
                            nc.vector.reduce_sum(
                                out=se_h[:, gtt:gtt + 1],
                                in_=acc_t[:],
                                axis=mybir.AxisListType.X,
                            )
                    se_halves.append(se_h)
                se_t = out_pool.tile([128, ttiles], f32, tag="se_t")
                nc.vector.tensor_add(se_t[:], se_halves[0][:], se_halves[1][:])
                nc.sync.dma_start(se.ap()[m], se_t[:])

                # selected-token logits: row dot products over the token shard
                sel_t = out_pool.tile([128, TSH // 128], f32, tag="sel_t")
                for st in range(TSH // 128):
                    xs_t = sel_pool.tile([128, H], f32, tag="selx")
                    ws_t = sel_pool.tile([128, H], f32, tag="selw")
                    nc.sync.dma_start(xs_t[:], xs_d.ap()[st * 128:(st + 1) * 128, :])
                    nc.sync.dma_start(ws_t[:], ws_d.ap()[st * 128:(st + 1) * 128, :])
                    pr_t = sel_pool.tile([128, H], f32, tag="selpr")
                    nc.vector.scalar_tensor_tensor(
                        out=pr_t[:],
                        in0=xs_t[:],
                        scalar=1.0,
                        in1=ws_t[:],
                        op0=mybir.AluOpType.mult,
                        op1=mybir.AluOpType.mult,
                        accum_out=sel_t[:, st:st + 1],
                    )
                nc.sync.dma_start(sel.ap()[m], sel_t[:])

    nc.compile()
    _nc_cache[mm_dtype] = nc
    return nc


def _prep_in_maps(inputs, mm_dtype=MM_DTYPE):
    import ml_dtypes

    x = np.ascontiguousarray(
        np.asarray(inputs["_input"], dtype=np.float32).reshape(TOK, H))
    rx = np.ascontiguousarray(
        np.asarray(inputs["ref_input"], dtype=np.float32).reshape(TOK, H))
    w = np.ascontiguousarray(np.asarray(inputs["lin_weight"], dtype=np.float32))
    rw = np.ascontiguousarray(np.asarray(inputs["ref_weight"], dtype=np.float32))
    ids = np.asarray(inputs["selected_token_ids"]).astype(np.int64).reshape(TOK)

    mmnp = np.float32 if mm_dtype == "f32r" else ml_dtypes.bfloat16
    xT = np.ascontiguousarray(x.T).astype(mmnp)
    rxT = np.ascontiguousarray(rx.T).astype(mmnp)

    wsel = w[ids]     # [TOK, H]
    rwsel = rw[ids]

    in_maps = []
    for c in range(NCORE):
        lo = c * VP
        real = max(0, min(lo + VP, V) - lo)
        wTc = np.zeros((H, VP), mmnp)
        rwTc = np.zeros((H, VP), mmnp)
        if real:
            wTc[:, :real] = w[lo:lo + real].T.astype(mmnp)
            rwTc[:, :real] = rw[lo:lo + real].T.astype(mmnp)
        tl = c * TSH
        in_maps.append({
            "xT": xT, "rxT": rxT, "wT": wTc, "rwT": rwTc,
            "xs": x[tl:tl + TSH], "rxs": rx[tl:tl + TSH],
            "ws": wsel[tl:tl + TSH], "rws": rwsel[tl:tl + TSH],
        })
    return in_maps


def _combine(results, inputs):
    """Host-side epilogue: logsumexp across shards + loss formula."""
    att = np.asarray(inputs["attention_mask"], dtype=np.float32)
    adv = np.asarray(inputs["advantages"], dtype=np.float32)

    se = np.stack([np.asarray(r["se"]) for r in results])    # [8, 2, 128, 32]
    sl = np.stack([np.asarray(r["sel"]) for r in results])   # [8, 2, 128, 4]

    # token t = t_tile*128 + p  ->  [core, m, t]
    se_tok = se.transpose(0, 1, 3, 2).reshape(NCORE, 2, TOK)
    npad = NCORE * VP - V          # 768 zero-pad vocab rows contribute exp(0)=1
    sumexp = se_tok.sum(axis=0, dtype=np.float32) - np.float32(npad)  # [2, TOK]

    # sel: global token = c*TSH + st*128 + p
    sel_tok = sl.transpose(1, 0, 3, 2).reshape(2, TOK)

    lp = (sel_tok[0] - np.log(sumexp[0])).astype(np.float32).reshape(B, T)
    rlp = (sel_tok[1] - np.log(sumexp[1])).astype(np.float32).reshape(B, T)

    # token-level IS ratio: exp(lp - stop_grad(lp)) == 1.0 exactly
    coef_1 = np.ones((B, T), dtype=np.float32)
    adv_b = adv[:, None]
    per_token_loss = -np.minimum(coef_1 * adv_b, coef_1 * adv_b)

    # k3 percentile KL
    flat = rlp.reshape(-1)
    k = max(1, int(flat.shape[0] * KL_PERCENTILE))
    threshold = np.sort(flat)[k - 1]
    mask = (rlp <= threshold).astype(np.float32)
    log_ratio = rlp - lp
    k3 = np.exp(log_ratio) - log_ratio - np.float32(1.0)
    scale = np.float32(1.0 / KL_PERCENTILE)
    kl_div = mask * k3 * scale

    per_token_loss = per_token_loss + np.float32(BETA) * kl_div

    normalizer = np.clip(att.sum(dtype=np.float32), 1.0, None).astype(np.float32)
    loss = (per_token_loss * att).sum(dtype=np.float32) / normalizer
    kl_metric = (kl_div * att).sum(dtype=np.float32) / normalizer
    is_clipped = ((coef_1 < 1.0 - EPS_LOW) & (adv_b < 0)) | (
        (coef_1 > 1.0 + EPS_HIGH) & (adv_b > 0))
    clip_ratio = (is_clipped.astype(np.float32) * att).sum(
        dtype=np.float32) / normalizer

    return (np.float32(loss), np.float32(kl_metric), np.float32(clip_ratio))


def kernel(**inputs):
    nc = build_nc()
    in_maps = _prep_in_maps(inputs)
    res = run_bass_kernel_spmd(nc, in_maps, core_ids=list(range(NCORE)))
    return _combine(res.results, inputs)
